# revision 23
# baseline (speedup 1.0000x reference)
"""Trainium2 Bass kernel for nn_BrainRegion (liquid-gated recurrent cell).

Computes, for full inputs (B=8192, IN=H=2048):
    xin  = concat([x_t, state], -1)
    cand = tanh(xin @ Wc + state @ Uc + bc)
    gate = sigmoid(xin @ Wg + state @ Ug + bg)
    alpha = exp(-1/exp(log_step))
    h    = alpha * state + (1 - alpha) * gate * cand
    out  = layernorm(h) * gamma + beta

Strategy: data-parallel over batch across 8 NeuronCores (1024 rows/core),
weights replicated.  Algebraic fold: xin@Wc + state@Uc == x_t@Wc[:IN] +
state@(Wc[IN:] + Uc), which removes one third of the FLOPs.  Mixed
precision: the tanh (cand) path runs in bf16 (its error passes through
tanh' up to 1.0); the sigmoid (gate) path runs in fp8 e4m3 with
DoubleRow perf mode (2x tensor throughput; sigmoid' <= 0.25 compresses
the quantization error).  PSUM accumulates in fp32; the elementwise
epilogue + layernorm run on-device in fp32; h/state/output in bf16.
"""

import sys

if "/opt/trn_rl_repo" not in sys.path:
    sys.path.insert(0, "/opt/trn_rl_repo")

import numpy as np
import ml_dtypes

B, IN, H = 8192, 2048, 2048
NCORES = 8
BC = B // NCORES      # rows per core (1024)
P = 128               # partitions
G = BC // P           # batch groups per core (8)
NJ = 8                # H slices for cand/epilogue
NSL = H // NJ         # slice width (256)
NJG = 4               # H slices for the fp8 gate matmuls
NGL = H // NJG        # gate slice width (512)
KT = H // P           # k-tiles per matrix (16)
KP = KT // 2          # fp8 DoubleRow k-pairs (8)
EPS = 1e-5

bf16 = ml_dtypes.bfloat16
e4m3 = ml_dtypes.float8_e4m3
SX = 16.0             # gate activation quant scale
SW = 256.0            # gate weight quant scale
DESCALE = 1.0 / (SX * SW)

# Set by test.py to collect a hardware profile.
TRACE = False
LAST_RESULTS = None

_compiled = {}


ALPHA0 = float(np.exp(-1.0))  # alpha when log_step == 0


def _build(flags):
    """Trace + compile the SPMD device program. flags = (has_bc, has_bg,
    has_gamma, has_beta, has_logstep) selects optional elementwise
    passes."""
    from contextlib import ExitStack

    import concourse.bass as bass
    import concourse.tile as tile
    from concourse import bacc, mybir

    has_bc, has_bg, has_gamma, has_beta, has_logstep = flags
    f32 = mybir.dt.float32
    bft = mybir.dt.bfloat16
    f8 = mybir.dt.float8e4
    DR = mybir.MatmulPerfMode.DoubleRow
    AF = mybir.ActivationFunctionType
    OP = mybir.AluOpType

    nc = bacc.Bacc("TRN2", target_bir_lowering=False, debug=False,
                   num_devices=NCORES)

    # DRAM I/O. Activation tensors are pre-arranged on host so every DMA
    # below is contiguous:
    #   xb4/sb4: [G, P, KT, P]   bf16, [g,p,k,m] = x[g*128+m, k*128+p]
    #   xq4/sq4: [G, P, KT, P]   fp8 (x*SX), same arrangement
    #   wc*:     [NJ, P, KT, NSL] bf16, [j,p,k,n] = W[k*128+p, j*NSL+n]
    #   wg*:     [NJG, P, KT, NGL] fp8 (W*SW), same with 512-wide slices
    xb4 = nc.dram_tensor("xb4", [G, P, KT, P], bft, kind="ExternalInput").ap()
    sb4 = nc.dram_tensor("sb4", [G, P, KT, P], bft, kind="ExternalInput").ap()
    xq4 = nc.dram_tensor("xq4", [G, P, KT, P], f8, kind="ExternalInput").ap()
    sq4 = nc.dram_tensor("sq4", [G, P, KT, P], f8, kind="ExternalInput").ap()
    stb = nc.dram_tensor("stb", [BC, H], bft, kind="ExternalInput").ap()
    wcx = nc.dram_tensor("wcx", [NJG, P, KT, NGL], bft,
                         kind="ExternalInput").ap()
    wcs = nc.dram_tensor("wcs", [NJG, P, KT, NGL], bft,
                         kind="ExternalInput").ap()
    wgx = nc.dram_tensor("wgx", [NJG, P, KT, NGL], f8,
                         kind="ExternalInput").ap()
    wgs = nc.dram_tensor("wgs", [NJG, P, KT, NGL], f8,
                         kind="ExternalInput").ap()
    if has_logstep:
        logb = nc.dram_tensor("logb", [P, H], f32,
                              kind="ExternalInput").ap()
    vecs = {}
    for name, used in (("bcb", has_bc), ("bgb", has_bg),
                       ("gammab", has_gamma), ("betab", has_beta)):
        if used:
            vecs[name] = nc.dram_tensor(name, [P, H], f32,
                                        kind="ExternalInput").ap()
    out = nc.dram_tensor("out", [BC, H], bft, kind="ExternalOutput").ap()

    with tile.TileContext(nc) as tc, ExitStack() as ctx:
        singles = ctx.enter_context(tc.tile_pool(name="singles", bufs=1))
        gactp = ctx.enter_context(tc.tile_pool(name="gactp", bufs=1))
        cactp = ctx.enter_context(tc.tile_pool(name="cactp", bufs=2))
        wcp = ctx.enter_context(tc.tile_pool(name="wcp", bufs=2))
        wgp = ctx.enter_context(tc.tile_pool(name="wgp", bufs=2))
        psgp = ctx.enter_context(tc.tile_pool(name="psgp", bufs=2,
                                              space="PSUM"))
        pscp = ctx.enter_context(tc.tile_pool(name="pscp", bufs=3,
                                              space="PSUM"))
        epp = ctx.enter_context(tc.tile_pool(name="epp", bufs=2))
        stp = ctx.enter_context(tc.tile_pool(name="stp", bufs=3))
        hp = ctx.enter_context(tc.tile_pool(name="hp", bufs=1))
        statp = ctx.enter_context(tc.tile_pool(name="statp", bufs=1))
        normp = ctx.enter_context(tc.tile_pool(name="normp", bufs=4))
        outp = ctx.enter_context(tc.tile_pool(name="outp", bufs=2))

        # ---- gate fp8 activations: resident for the whole kernel.
        # DMA'd lazily inside the first jg sweep so the first weight
        # slices aren't stuck behind 4 MB of activation DMA.
        xq_t = [gactp.tile([P, KT, P], f8, name=f"xq_g{g}", tag=f"xq{g}")
                for g in range(G)]
        sq_t = [gactp.tile([P, KT, P], f8, name=f"sq_g{g}", tag=f"sq{g}")
                for g in range(G)]

        # ---- constants: alpha = exp(-exp(-log_step)), broadcast [P, H].
        # When log_step == 0 (has_logstep False) alpha is the compile-time
        # scalar ALPHA0 and no tile is needed.
        if has_logstep:
            alpha_t = singles.tile([P, H], f32, name="alpha_t")
            nc.sync.dma_start(out=alpha_t[:], in_=logb[:])
            nc.scalar.activation(alpha_t[:], alpha_t[:], AF.Exp, scale=-1.0)
            nc.scalar.activation(alpha_t[:], alpha_t[:], AF.Exp, scale=-1.0)
        eps_t = singles.tile([P, 1], f32, name="eps_t")
        nc.vector.memset(eps_t[:], EPS)
        vt = {}
        for name in vecs:
            vt[name] = singles.tile([P, H], f32, name=name + "_t")
            nc.sync.dma_start(out=vt[name][:], in_=vecs[name][:])

        # ---- per-group h accumulator (bf16) and layernorm stats ----
        h_t = [hp.tile([P, H], bft, name=f"h_g{g}", tag=f"h{g}")
               for g in range(G)]
        stats_t = [statp.tile([P, NJ, 6], f32, name=f"stats_g{g}",
                              tag=f"st{g}")
                   for g in range(G)]

        # ---- main loops: jg = gate H slice (2 cand slices), g = batch ----
        # Weight tiles are split into k-halves so the first matmuls can
        # start after ~0.5 MB of DMA instead of ~4 MB; DMA issue order at
        # jg=0 follows the order the tensor engine consumes the bytes.
        KH = KT // 2

        def wload(pool, dram, jg, nm, dt, defer):
            a = pool.tile([P, KH, NGL], dt, name=f"{nm}a_{jg}", tag=nm + "a")
            b = pool.tile([P, KH, NGL], dt, name=f"{nm}b_{jg}", tag=nm + "b")
            nc.sync.dma_start(out=a[:], in_=dram[jg, :, 0:KH])
            if not defer:
                nc.sync.dma_start(out=b[:], in_=dram[jg, :, KH:KT])
            return a, b

        def wslice(ab, k, width=1):
            t = ab[k // KH]
            lo = k % KH
            return t[:, lo:lo + width, :] if width > 1 else t[:, lo, :]

        prefetched = {}
        for jg in range(NJG):
            first = jg == 0
            wgx_t = wload(wgp, wgx, jg, "wgx", f8, first)
            if first:
                # first matmul needs only wgx[0][k<8] + g0 activations
                nc.sync.dma_start(out=xq_t[0][:], in_=xq4[0])
                nc.sync.dma_start(out=sq_t[0][:], in_=sq4[0])
                xb0 = cactp.tile([P, KT, P], bft, name="xb_0_0", tag="xb")
                sb0 = cactp.tile([P, KT, P], bft, name="sb_0_0", tag="sb")
                nc.sync.dma_start(out=xb0[:], in_=xb4[0])
                nc.sync.dma_start(out=sb0[:], in_=sb4[0])
                prefetched[(0, 0)] = (xb0, sb0)
            wgs_t = wload(wgp, wgs, jg, "wgs", f8, first)
            if first:
                nc.sync.dma_start(out=wgx_t[1][:], in_=wgx[0, :, KH:KT])
            wcx_t = wload(wcp, wcx, jg, "wcx", bft, first)
            wcs_t = wload(wcp, wcs, jg, "wcs", bft, first)
            if first:
                nc.sync.dma_start(out=wgs_t[1][:], in_=wgs[0, :, KH:KT])
                nc.sync.dma_start(out=wcx_t[1][:], in_=wcx[0, :, KH:KT])
                nc.sync.dma_start(out=wcs_t[1][:], in_=wcs[0, :, KH:KT])

            for g in range(G):
                if jg == 0 and g > 0:
                    nc.sync.dma_start(out=xq_t[g][:], in_=xq4[g])
                    nc.sync.dma_start(out=sq_t[g][:], in_=sq4[g])
                # stream the cand bf16 activations for this (jg, g)
                if (jg, g) in prefetched:
                    xb_t, sb_t = prefetched.pop((jg, g))
                else:
                    xb_t = cactp.tile([P, KT, P], bft, name=f"xb_{jg}_{g}",
                                      tag="xb")
                    sb_t = cactp.tile([P, KT, P], bft, name=f"sb_{jg}_{g}",
                                      tag="sb")
                    nc.sync.dma_start(out=xb_t[:], in_=xb4[g])
                    nc.sync.dma_start(out=sb_t[:], in_=sb4[g])

                # gate: fp8 DoubleRow matmuls, 512-wide moving stream
                pg = psgp.tile([P, NGL], f32, name=f"pg_{jg}_{g}", tag="pg")
                for kp in range(KP):
                    ks = slice(2 * kp, 2 * kp + 2)
                    nc.tensor.matmul(pg[:], xq_t[g][:, ks, :],
                                     wslice(wgx_t, 2 * kp, 2),
                                     start=(kp == 0), stop=False,
                                     perf_mode=DR)
                for kp in range(KP):
                    ks = slice(2 * kp, 2 * kp + 2)
                    nc.tensor.matmul(pg[:], sq_t[g][:, ks, :],
                                     wslice(wgs_t, 2 * kp, 2),
                                     start=False, stop=(kp == KP - 1),
                                     perf_mode=DR)

                # cand: bf16 matmuls, 512-wide moving stream
                pc = pscp.tile([P, NGL], f32, name=f"pc_{jg}_{g}",
                               tag="pc")
                for k in range(KT):
                    nc.tensor.matmul(pc[:], xb_t[:, k, :],
                                     wslice(wcx_t, k),
                                     start=(k == 0), stop=False)
                for k in range(KT):
                    nc.tensor.matmul(pc[:], sb_t[:, k, :],
                                     wslice(wcs_t, k),
                                     start=False, stop=(k == KT - 1))

                for j in (2 * jg, 2 * jg + 1):
                    jsl = slice(j * NSL, (j + 1) * NSL)
                    off = (j - 2 * jg) * NSL
                    pcs = pc[:, off:off + NSL]
                    pgs = pg[:, off:off + NSL]

                    # epilogue for this (g, j) slice
                    sc = epp.tile([P, NSL], f32, name=f"sc_{j}_{g}",
                                  tag="sc")
                    sg = epp.tile([P, NSL], f32, name=f"sg_{j}_{g}",
                                  tag="sg")
                    if has_bc:
                        nc.vector.scalar_tensor_tensor(
                            sc[:], pcs, 1.0, vt["bcb"][:, jsl],
                            op0=OP.mult, op1=OP.add)
                        nc.scalar.activation(sc[:], sc[:], AF.Tanh)
                    else:
                        nc.scalar.activation(sc[:], pcs, AF.Tanh)
                    if has_bg:
                        nc.vector.scalar_tensor_tensor(
                            sg[:], pgs, DESCALE, vt["bgb"][:, jsl],
                            op0=OP.mult, op1=OP.add)
                        nc.scalar.activation(sg[:], sg[:], AF.Sigmoid)
                    else:
                        nc.scalar.activation(sg[:], pgs, AF.Sigmoid,
                                             scale=DESCALE)

                    st_sl = stp.tile([P, NSL], bft, name=f"stsl_{j}_{g}",
                                     tag="stsl")
                    nc.sync.dma_start(
                        out=st_sl[:],
                        in_=stb[g * P:(g + 1) * P, jsl])

                    # h = gc + alpha*(state - gc), with gc = gate*cand
                    t2 = epp.tile([P, NSL], f32, name=f"t2_{j}_{g}",
                                  tag="t2")
                    nc.vector.tensor_mul(t2[:], sc[:], sg[:])   # gate*cand
                    t3 = epp.tile([P, NSL], f32, name=f"t3_{j}_{g}",
                                  tag="t3")
                    nc.vector.tensor_sub(t3[:], st_sl[:], t2[:])
                    hsl = h_t[g][:, jsl]
                    if has_logstep:
                        nc.vector.tensor_mul(t3[:], t3[:], alpha_t[:, jsl])
                        nc.vector.tensor_add(out=hsl, in0=t2[:], in1=t3[:])
                    else:
                        # h = t2 + ALPHA0 * t3, written to bf16 h_t
                        nc.vector.scalar_tensor_tensor(
                            hsl, t3[:], ALPHA0, t2[:],
                            op0=OP.mult, op1=OP.add)

                    nc.vector.bn_stats(out=stats_t[g][:, j, :], in_=hsl)

                    if j == NJ - 1:
                        # layernorm + output for this group, overlapping
                        # the remaining groups' matmuls
                        mv = normp.tile([P, 2], f32, name=f"mv_{g}",
                                        tag="mv")
                        nc.vector.bn_aggr(out=mv[:], in_=stats_t[g][:])
                        rstd = normp.tile([P, 1], f32, name=f"rstd_{g}",
                                          tag="rstd")
                        nc.scalar.activation(rstd[:], mv[:, 1:2], AF.Sqrt,
                                             bias=eps_t[:])
                        nc.vector.reciprocal(rstd[:], rstd[:])
                        ot = outp.tile([P, H], bft, name=f"ot_{g}",
                                       tag="ot")
                        HH = H // 2
                        for half in range(2):
                            hs = slice(half * HH, (half + 1) * HH)
                            nc.vector.tensor_scalar(
                                ot[:, hs], h_t[g][:, hs],
                                mv[:, 0:1], rstd[:],
                                op0=OP.subtract, op1=OP.mult)
                            if has_gamma:
                                nc.vector.tensor_mul(ot[:, hs], ot[:, hs],
                                                     vt["gammab"][:, hs])
                            if has_beta:
                                nc.vector.tensor_add(ot[:, hs], ot[:, hs],
                                                     vt["betab"][:, hs])
                            nc.sync.dma_start(
                                out=out[g * P:(g + 1) * P, hs],
                                in_=ot[:, hs])

    nc.compile()
    return nc


def _get_compiled(flags):
    if flags not in _compiled:
        _compiled[flags] = _build(flags)
    return _compiled[flags]


def kernel(x_t, state, Wc, Uc, bc, Wg, Ug, bg, log_step, gamma, beta):
    global LAST_RESULTS
    from concourse import bass_utils

    x_t = np.asarray(x_t, np.float32)
    state = np.asarray(state, np.float32)
    Wc = np.asarray(Wc, np.float32)
    Uc = np.asarray(Uc, np.float32)
    Wg = np.asarray(Wg, np.float32)
    Ug = np.asarray(Ug, np.float32)
    bc = np.asarray(bc, np.float32)
    bg = np.asarray(bg, np.float32)
    log_step = np.asarray(log_step, np.float32)
    gamma = np.asarray(gamma, np.float32)
    beta = np.asarray(beta, np.float32)

    # fold the recurrent weights and pre-tile for the device:
    # [j, p, k, n] = W[k*128+p, j*W_SL+n]
    def wtile(w, dt, scale, nj, nsl):
        return np.ascontiguousarray(
            (w * scale).astype(dt).reshape(KT, P, nj, nsl)
            .transpose(2, 1, 0, 3))

    Wcs_f = Wc[IN:] + Uc
    Wgs_f = Wg[IN:] + Ug
    w_maps = {
        "wcx": wtile(Wc[:IN], bf16, 1.0, NJG, NGL),
        "wcs": wtile(Wcs_f, bf16, 1.0, NJG, NGL),
        "wgx": wtile(Wg[:IN], e4m3, SW, NJG, NGL),
        "wgs": wtile(Wgs_f, e4m3, SW, NJG, NGL),
    }
    flags = (bool(bc.any()), bool(bg.any()),
             bool((gamma != 1.0).any()), bool(beta.any()),
             bool(log_step.any()))
    vec_maps = {}
    if flags[4]:
        vec_maps["logb"] = np.ascontiguousarray(
            np.broadcast_to(log_step.reshape(1, H), (P, H)))
    if flags[0]:
        vec_maps["bcb"] = np.ascontiguousarray(
            np.broadcast_to(bc.reshape(1, H), (P, H)))
    if flags[1]:
        vec_maps["bgb"] = np.ascontiguousarray(
            np.broadcast_to(bg.reshape(1, H), (P, H)))
    if flags[2]:
        vec_maps["gammab"] = np.ascontiguousarray(
            np.broadcast_to(gamma.reshape(1, H), (P, H)))
    if flags[3]:
        vec_maps["betab"] = np.ascontiguousarray(
            np.broadcast_to(beta.reshape(1, H), (P, H)))

    nc = _get_compiled(flags)

    # per-core activation shards, pre-tiled: [g, p, k, m] = x[g*128+m, k*128+p]
    def atile(a, dt, scale):
        return np.ascontiguousarray(
            (a * scale).astype(dt).reshape(G, P, KT, P).transpose(0, 3, 2, 1))

    in_maps = []
    for c in range(NCORES):
        rows = slice(c * BC, (c + 1) * BC)
        m = {
            "xb4": atile(x_t[rows], bf16, 1.0),
            "sb4": atile(state[rows], bf16, 1.0),
            "xq4": atile(x_t[rows], e4m3, SX),
            "sq4": atile(state[rows], e4m3, SX),
            "stb": np.ascontiguousarray(state[rows].astype(bf16)),
        }
        m.update(w_maps)
        m.update(vec_maps)
        in_maps.append(m)

    trace_kwargs = {}
    if TRACE:
        trace_kwargs["trace_cores"] = list(range(NCORES))
    res = bass_utils.run_bass_kernel_spmd(
        nc, in_maps, core_ids=list(range(NCORES)), trace=TRACE,
        **trace_kwargs)
    LAST_RESULTS = res
    return np.concatenate(
        [res.results[c]["out"] for c in range(NCORES)],
        axis=0).astype(np.float32)


# revision 25
# speedup vs baseline: 1.0095x; 1.0095x over previous
"""Trainium2 Bass kernel for nn_BrainRegion (liquid-gated recurrent cell).

Computes, for full inputs (B=8192, IN=H=2048):
    xin  = concat([x_t, state], -1)
    cand = tanh(xin @ Wc + state @ Uc + bc)
    gate = sigmoid(xin @ Wg + state @ Ug + bg)
    alpha = exp(-1/exp(log_step))
    h    = alpha * state + (1 - alpha) * gate * cand
    out  = layernorm(h) * gamma + beta

Strategy: data-parallel over batch across 8 NeuronCores (1024 rows/core),
weights replicated.  Algebraic fold: xin@Wc + state@Uc == x_t@Wc[:IN] +
state@(Wc[IN:] + Uc), which removes one third of the FLOPs.  Mixed
precision: the tanh (cand) path runs in bf16 (its error passes through
tanh' up to 1.0); the sigmoid (gate) path runs in fp8 e4m3 with
DoubleRow perf mode (2x tensor throughput; sigmoid' <= 0.25 compresses
the quantization error).  PSUM accumulates in fp32; the elementwise
epilogue + layernorm run on-device in fp32; h/state/output in bf16.
"""

import sys

if "/opt/trn_rl_repo" not in sys.path:
    sys.path.insert(0, "/opt/trn_rl_repo")

import numpy as np
import ml_dtypes

B, IN, H = 8192, 2048, 2048
NCORES = 8
BC = B // NCORES      # rows per core (1024)
P = 128               # partitions
G = BC // P           # batch groups per core (8)
NJ = 8                # H slices for cand/epilogue
NSL = H // NJ         # slice width (256)
NJG = 4               # H slices for the fp8 gate matmuls
NGL = H // NJG        # gate slice width (512)
KT = H // P           # k-tiles per matrix (16)
KP = KT // 2          # fp8 DoubleRow k-pairs (8)
EPS = 1e-5

bf16 = ml_dtypes.bfloat16
e4m3 = ml_dtypes.float8_e4m3
SX = 16.0             # gate activation quant scale
SW = 256.0            # gate weight quant scale
DESCALE = 1.0 / (SX * SW)

# Set by test.py to collect a hardware profile.
TRACE = False
LAST_RESULTS = None

_compiled = {}


ALPHA0 = float(np.exp(-1.0))  # alpha when log_step == 0


def _build(flags):
    """Trace + compile the SPMD device program. flags = (has_bc, has_bg,
    has_gamma, has_beta, has_logstep) selects optional elementwise
    passes."""
    from contextlib import ExitStack

    import concourse.bass as bass
    import concourse.tile as tile
    from concourse import bacc, mybir

    has_bc, has_bg, has_gamma, has_beta, has_logstep = flags
    f32 = mybir.dt.float32
    bft = mybir.dt.bfloat16
    f8 = mybir.dt.float8e4
    DR = mybir.MatmulPerfMode.DoubleRow
    AF = mybir.ActivationFunctionType
    OP = mybir.AluOpType

    nc = bacc.Bacc("TRN2", target_bir_lowering=False, debug=False,
                   num_devices=NCORES)

    # DRAM I/O. Activation tensors are pre-arranged on host so every DMA
    # below is contiguous:
    #   xb4/sb4: [G, P, KT, P]   bf16, [g,p,k,m] = x[g*128+m, k*128+p]
    #   xq4/sq4: [G, P, KT, P]   fp8 (x*SX), same arrangement
    #   wc*:     [NJ, P, KT, NSL] bf16, [j,p,k,n] = W[k*128+p, j*NSL+n]
    #   wg*:     [NJG, P, KT, NGL] fp8 (W*SW), same with 512-wide slices
    xb4 = nc.dram_tensor("xb4", [G, P, KT, P], bft, kind="ExternalInput").ap()
    sb4 = nc.dram_tensor("sb4", [G, P, KT, P], bft, kind="ExternalInput").ap()
    xq4 = nc.dram_tensor("xq4", [G, P, KT, P], f8, kind="ExternalInput").ap()
    sq4 = nc.dram_tensor("sq4", [G, P, KT, P], f8, kind="ExternalInput").ap()
    stb = nc.dram_tensor("stb", [BC, H], bft, kind="ExternalInput").ap()
    wcx = nc.dram_tensor("wcx", [NJG, P, KT, NGL], bft,
                         kind="ExternalInput").ap()
    wcs = nc.dram_tensor("wcs", [NJG, P, KT, NGL], bft,
                         kind="ExternalInput").ap()
    wgx = nc.dram_tensor("wgx", [NJG, P, KT, NGL], f8,
                         kind="ExternalInput").ap()
    wgs = nc.dram_tensor("wgs", [NJG, P, KT, NGL], f8,
                         kind="ExternalInput").ap()
    if has_logstep:
        logb = nc.dram_tensor("logb", [P, H], f32,
                              kind="ExternalInput").ap()
    vecs = {}
    for name, used in (("bcb", has_bc), ("bgb", has_bg),
                       ("gammab", has_gamma), ("betab", has_beta)):
        if used:
            vecs[name] = nc.dram_tensor(name, [P, H], f32,
                                        kind="ExternalInput").ap()
    out = nc.dram_tensor("out", [BC, H], bft, kind="ExternalOutput").ap()

    with tile.TileContext(nc) as tc, ExitStack() as ctx:
        singles = ctx.enter_context(tc.tile_pool(name="singles", bufs=1))
        gactp = ctx.enter_context(tc.tile_pool(name="gactp", bufs=1))
        cactp = ctx.enter_context(tc.tile_pool(name="cactp", bufs=2))
        wcp = ctx.enter_context(tc.tile_pool(name="wcp", bufs=2))
        wgp = ctx.enter_context(tc.tile_pool(name="wgp", bufs=2))
        # psgp needs SKEW+1 buffers: gate matmuls for group g+SKEW are
        # emitted before the epilogue that frees group g's psum tile.
        psgp = ctx.enter_context(tc.tile_pool(name="psgp", bufs=3,
                                              space="PSUM"))
        pscp = ctx.enter_context(tc.tile_pool(name="pscp", bufs=2,
                                              space="PSUM"))
        epp = ctx.enter_context(tc.tile_pool(name="epp", bufs=2))
        stp = ctx.enter_context(tc.tile_pool(name="stp", bufs=3))
        hp = ctx.enter_context(tc.tile_pool(name="hp", bufs=1))
        statp = ctx.enter_context(tc.tile_pool(name="statp", bufs=1))
        normp = ctx.enter_context(tc.tile_pool(name="normp", bufs=4))
        outp = ctx.enter_context(tc.tile_pool(name="outp", bufs=2))

        # ---- gate fp8 activations: resident for the whole kernel.
        # DMA'd lazily inside the first jg sweep so the first weight
        # slices aren't stuck behind 4 MB of activation DMA.
        xq_t = [gactp.tile([P, KT, P], f8, name=f"xq_g{g}", tag=f"xq{g}")
                for g in range(G)]
        sq_t = [gactp.tile([P, KT, P], f8, name=f"sq_g{g}", tag=f"sq{g}")
                for g in range(G)]

        # ---- constants: alpha = exp(-exp(-log_step)), broadcast [P, H].
        # When log_step == 0 (has_logstep False) alpha is the compile-time
        # scalar ALPHA0 and no tile is needed.
        if has_logstep:
            alpha_t = singles.tile([P, H], f32, name="alpha_t")
            nc.sync.dma_start(out=alpha_t[:], in_=logb[:])
            nc.scalar.activation(alpha_t[:], alpha_t[:], AF.Exp, scale=-1.0)
            nc.scalar.activation(alpha_t[:], alpha_t[:], AF.Exp, scale=-1.0)
        eps_t = singles.tile([P, 1], f32, name="eps_t")
        nc.vector.memset(eps_t[:], EPS)
        vt = {}
        for name in vecs:
            vt[name] = singles.tile([P, H], f32, name=name + "_t")
            nc.sync.dma_start(out=vt[name][:], in_=vecs[name][:])

        # ---- per-group h accumulator (bf16) and layernorm stats ----
        h_t = [hp.tile([P, H], bft, name=f"h_g{g}", tag=f"h{g}")
               for g in range(G)]
        stats_t = [statp.tile([P, NJ, 6], f32, name=f"stats_g{g}",
                              tag=f"st{g}")
                   for g in range(G)]

        # ---- main loops: jg = gate H slice (2 cand slices), g = batch ----
        # Weight tiles are split into k-halves so the first matmuls can
        # start after ~0.5 MB of DMA instead of ~4 MB; DMA issue order at
        # jg=0 follows the order the tensor engine consumes the bytes.
        KH = KT // 2

        def wload(pool, dram, jg, nm, dt, defer):
            a = pool.tile([P, KH, NGL], dt, name=f"{nm}a_{jg}", tag=nm + "a")
            b = pool.tile([P, KH, NGL], dt, name=f"{nm}b_{jg}", tag=nm + "b")
            nc.sync.dma_start(out=a[:], in_=dram[jg, :, 0:KH])
            if not defer:
                nc.sync.dma_start(out=b[:], in_=dram[jg, :, KH:KT])
            return a, b

        def wslice(ab, k, width=1):
            t = ab[k // KH]
            lo = k % KH
            return t[:, lo:lo + width, :] if width > 1 else t[:, lo, :]

        # PE clock warmup: tiny matmuls on zeroed tiles keep the tensor
        # engine busy during the initial DMA head so the real matmuls
        # start at full clock instead of ramping from the low p-state.
        warm_s = singles.tile([P, P], bft, name="warm_s")
        warm_m = singles.tile([P, P], bft, name="warm_m")
        nc.vector.memset(warm_s[:], 0.0)
        nc.vector.memset(warm_m[:], 0.0)
        warmpp = ctx.enter_context(tc.tile_pool(name="warmpp", bufs=2,
                                                space="PSUM"))
        for i in range(64):
            wps = warmpp.tile([P, P], f32, name=f"warm_p{i}", tag="warmp")
            nc.tensor.matmul(wps[:], warm_s[:], warm_m[:],
                             start=True, stop=True)

        prefetched = {}
        SKEW = 2
        pg_live = {}

        def emit_gate(jg, g, wgx_t, wgs_t):
            if jg == 0 and g > 0:
                nc.sync.dma_start(out=xq_t[g][:], in_=xq4[g])
                nc.sync.dma_start(out=sq_t[g][:], in_=sq4[g])
            # gate: fp8 DoubleRow matmuls, 512-wide moving stream
            pg = psgp.tile([P, NGL], f32, name=f"pg_{jg}_{g}", tag="pg")
            for kp in range(KP):
                ks = slice(2 * kp, 2 * kp + 2)
                nc.tensor.matmul(pg[:], xq_t[g][:, ks, :],
                                 wslice(wgx_t, 2 * kp, 2),
                                 start=(kp == 0), stop=False,
                                 perf_mode=DR)
            for kp in range(KP):
                ks = slice(2 * kp, 2 * kp + 2)
                nc.tensor.matmul(pg[:], sq_t[g][:, ks, :],
                                 wslice(wgs_t, 2 * kp, 2),
                                 start=False, stop=(kp == KP - 1),
                                 perf_mode=DR)
            pg_live[(jg, g)] = pg

        for jg in range(NJG):
            first = jg == 0
            wgx_t = wload(wgp, wgx, jg, "wgx", f8, first)
            if first:
                # first matmul needs only wgx[0][k<8] + g0 activations
                nc.sync.dma_start(out=xq_t[0][:], in_=xq4[0])
                nc.sync.dma_start(out=sq_t[0][:], in_=sq4[0])
            wgs_t = wload(wgp, wgs, jg, "wgs", f8, first)
            if first:
                xb0 = cactp.tile([P, KT, P], bft, name="xb_0_0", tag="xb")
                sb0 = cactp.tile([P, KT, P], bft, name="sb_0_0", tag="sb")
                nc.sync.dma_start(out=xb0[:], in_=xb4[0])
                nc.sync.dma_start(out=sb0[:], in_=sb4[0])
                prefetched[(0, 0)] = (xb0, sb0)
                nc.sync.dma_start(out=wgx_t[1][:], in_=wgx[0, :, KH:KT])
            wcx_t = wload(wcp, wcx, jg, "wcx", bft, first)
            wcs_t = wload(wcp, wcs, jg, "wcs", bft, first)
            if first:
                nc.sync.dma_start(out=wgs_t[1][:], in_=wgs[0, :, KH:KT])
                nc.sync.dma_start(out=wcx_t[1][:], in_=wcx[0, :, KH:KT])
                nc.sync.dma_start(out=wcs_t[1][:], in_=wcs[0, :, KH:KT])

            # software-pipelined emission: gate matmuls run SKEW batch
            # groups ahead of the cand matmuls, so the tensor queue has
            # gate work whenever cand weights/activations are still in
            # flight.
            for g in range(min(SKEW, G)):
                emit_gate(jg, g, wgx_t, wgs_t)

            for g in range(G):
                if g + SKEW < G:
                    emit_gate(jg, g + SKEW, wgx_t, wgs_t)
                # stream the cand bf16 activations for this (jg, g)
                if (jg, g) in prefetched:
                    xb_t, sb_t = prefetched.pop((jg, g))
                else:
                    xb_t = cactp.tile([P, KT, P], bft, name=f"xb_{jg}_{g}",
                                      tag="xb")
                    sb_t = cactp.tile([P, KT, P], bft, name=f"sb_{jg}_{g}",
                                      tag="sb")
                    nc.sync.dma_start(out=xb_t[:], in_=xb4[g])
                    nc.sync.dma_start(out=sb_t[:], in_=sb4[g])

                # cand: bf16 matmuls, 512-wide moving stream
                pc = pscp.tile([P, NGL], f32, name=f"pc_{jg}_{g}",
                               tag="pc")
                for k in range(KT):
                    nc.tensor.matmul(pc[:], xb_t[:, k, :],
                                     wslice(wcx_t, k),
                                     start=(k == 0), stop=False)
                for k in range(KT):
                    nc.tensor.matmul(pc[:], sb_t[:, k, :],
                                     wslice(wcs_t, k),
                                     start=False, stop=(k == KT - 1))
                pg = pg_live.pop((jg, g))

                for j in (2 * jg, 2 * jg + 1):
                    jsl = slice(j * NSL, (j + 1) * NSL)
                    off = (j - 2 * jg) * NSL
                    pcs = pc[:, off:off + NSL]
                    pgs = pg[:, off:off + NSL]

                    # epilogue for this (g, j) slice
                    sc = epp.tile([P, NSL], f32, name=f"sc_{j}_{g}",
                                  tag="sc")
                    sg = epp.tile([P, NSL], f32, name=f"sg_{j}_{g}",
                                  tag="sg")
                    if has_bc:
                        nc.vector.scalar_tensor_tensor(
                            sc[:], pcs, 1.0, vt["bcb"][:, jsl],
                            op0=OP.mult, op1=OP.add)
                        nc.scalar.activation(sc[:], sc[:], AF.Tanh)
                    else:
                        nc.scalar.activation(sc[:], pcs, AF.Tanh)
                    if has_bg:
                        nc.vector.scalar_tensor_tensor(
                            sg[:], pgs, DESCALE, vt["bgb"][:, jsl],
                            op0=OP.mult, op1=OP.add)
                        nc.scalar.activation(sg[:], sg[:], AF.Sigmoid)
                    else:
                        nc.scalar.activation(sg[:], pgs, AF.Sigmoid,
                                             scale=DESCALE)

                    st_sl = stp.tile([P, NSL], bft, name=f"stsl_{j}_{g}",
                                     tag="stsl")
                    nc.sync.dma_start(
                        out=st_sl[:],
                        in_=stb[g * P:(g + 1) * P, jsl])

                    # h = gc + alpha*(state - gc), with gc = gate*cand
                    t2 = epp.tile([P, NSL], f32, name=f"t2_{j}_{g}",
                                  tag="t2")
                    nc.vector.tensor_mul(t2[:], sc[:], sg[:])   # gate*cand
                    t3 = epp.tile([P, NSL], f32, name=f"t3_{j}_{g}",
                                  tag="t3")
                    nc.vector.tensor_sub(t3[:], st_sl[:], t2[:])
                    hsl = h_t[g][:, jsl]
                    if has_logstep:
                        nc.vector.tensor_mul(t3[:], t3[:], alpha_t[:, jsl])
                        nc.vector.tensor_add(out=hsl, in0=t2[:], in1=t3[:])
                    else:
                        # h = t2 + ALPHA0 * t3, written to bf16 h_t
                        nc.vector.scalar_tensor_tensor(
                            hsl, t3[:], ALPHA0, t2[:],
                            op0=OP.mult, op1=OP.add)

                    nc.vector.bn_stats(out=stats_t[g][:, j, :], in_=hsl)

                    if j == NJ - 1:
                        # layernorm + output for this group, overlapping
                        # the remaining groups' matmuls
                        mv = normp.tile([P, 2], f32, name=f"mv_{g}",
                                        tag="mv")
                        nc.vector.bn_aggr(out=mv[:], in_=stats_t[g][:])
                        rstd = normp.tile([P, 1], f32, name=f"rstd_{g}",
                                          tag="rstd")
                        nc.scalar.activation(rstd[:], mv[:, 1:2], AF.Sqrt,
                                             bias=eps_t[:])
                        nc.vector.reciprocal(rstd[:], rstd[:])
                        ot = outp.tile([P, H], bft, name=f"ot_{g}",
                                       tag="ot")
                        HH = H // 2
                        for half in range(2):
                            hs = slice(half * HH, (half + 1) * HH)
                            nc.vector.tensor_scalar(
                                ot[:, hs], h_t[g][:, hs],
                                mv[:, 0:1], rstd[:],
                                op0=OP.subtract, op1=OP.mult)
                            if has_gamma:
                                nc.vector.tensor_mul(ot[:, hs], ot[:, hs],
                                                     vt["gammab"][:, hs])
                            if has_beta:
                                nc.vector.tensor_add(ot[:, hs], ot[:, hs],
                                                     vt["betab"][:, hs])
                            nc.sync.dma_start(
                                out=out[g * P:(g + 1) * P, hs],
                                in_=ot[:, hs])

    nc.compile()
    return nc


def _get_compiled(flags):
    if flags not in _compiled:
        _compiled[flags] = _build(flags)
    return _compiled[flags]


def kernel(x_t, state, Wc, Uc, bc, Wg, Ug, bg, log_step, gamma, beta):
    global LAST_RESULTS
    from concourse import bass_utils

    x_t = np.asarray(x_t, np.float32)
    state = np.asarray(state, np.float32)
    Wc = np.asarray(Wc, np.float32)
    Uc = np.asarray(Uc, np.float32)
    Wg = np.asarray(Wg, np.float32)
    Ug = np.asarray(Ug, np.float32)
    bc = np.asarray(bc, np.float32)
    bg = np.asarray(bg, np.float32)
    log_step = np.asarray(log_step, np.float32)
    gamma = np.asarray(gamma, np.float32)
    beta = np.asarray(beta, np.float32)

    # fold the recurrent weights and pre-tile for the device:
    # [j, p, k, n] = W[k*128+p, j*W_SL+n]
    def wtile(w, dt, scale, nj, nsl):
        return np.ascontiguousarray(
            (w * scale).astype(dt).reshape(KT, P, nj, nsl)
            .transpose(2, 1, 0, 3))

    Wcs_f = Wc[IN:] + Uc
    Wgs_f = Wg[IN:] + Ug
    w_maps = {
        "wcx": wtile(Wc[:IN], bf16, 1.0, NJG, NGL),
        "wcs": wtile(Wcs_f, bf16, 1.0, NJG, NGL),
        "wgx": wtile(Wg[:IN], e4m3, SW, NJG, NGL),
        "wgs": wtile(Wgs_f, e4m3, SW, NJG, NGL),
    }
    flags = (bool(bc.any()), bool(bg.any()),
             bool((gamma != 1.0).any()), bool(beta.any()),
             bool(log_step.any()))
    vec_maps = {}
    if flags[4]:
        vec_maps["logb"] = np.ascontiguousarray(
            np.broadcast_to(log_step.reshape(1, H), (P, H)))
    if flags[0]:
        vec_maps["bcb"] = np.ascontiguousarray(
            np.broadcast_to(bc.reshape(1, H), (P, H)))
    if flags[1]:
        vec_maps["bgb"] = np.ascontiguousarray(
            np.broadcast_to(bg.reshape(1, H), (P, H)))
    if flags[2]:
        vec_maps["gammab"] = np.ascontiguousarray(
            np.broadcast_to(gamma.reshape(1, H), (P, H)))
    if flags[3]:
        vec_maps["betab"] = np.ascontiguousarray(
            np.broadcast_to(beta.reshape(1, H), (P, H)))

    nc = _get_compiled(flags)

    # per-core activation shards, pre-tiled: [g, p, k, m] = x[g*128+m, k*128+p]
    def atile(a, dt, scale):
        return np.ascontiguousarray(
            (a * scale).astype(dt).reshape(G, P, KT, P).transpose(0, 3, 2, 1))

    in_maps = []
    for c in range(NCORES):
        rows = slice(c * BC, (c + 1) * BC)
        m = {
            "xb4": atile(x_t[rows], bf16, 1.0),
            "sb4": atile(state[rows], bf16, 1.0),
            "xq4": atile(x_t[rows], e4m3, SX),
            "sq4": atile(state[rows], e4m3, SX),
            "stb": np.ascontiguousarray(state[rows].astype(bf16)),
        }
        m.update(w_maps)
        m.update(vec_maps)
        in_maps.append(m)

    trace_kwargs = {}
    if TRACE:
        trace_kwargs["trace_cores"] = list(range(NCORES))
    res = bass_utils.run_bass_kernel_spmd(
        nc, in_maps, core_ids=list(range(NCORES)), trace=TRACE,
        **trace_kwargs)
    LAST_RESULTS = res
    return np.concatenate(
        [res.results[c]["out"] for c in range(NCORES)],
        axis=0).astype(np.float32)
